# revision 5
# baseline (speedup 1.0000x reference)
import sys

sys.path.insert(0, "/opt/trn_rl_repo")
import numpy as np
from contextlib import ExitStack

from concourse import bacc
import concourse.tile as tile
from concourse import mybir
from concourse.bass_utils import run_bass_kernel_spmd

fp32 = mybir.dt.float32
fp32r = mybir.dt.float32r
Exp = mybir.ActivationFunctionType.Exp

B, S, HID = 4, 2048, 1024
H, DK = 16, 64
SQC = 1024  # query rows per core (2 cores per batch)
NPAIR = 8   # head pairs of 128 hidden dims

_PROG = None


def _build_program():
    nc = bacc.Bacc("TRN2", target_bir_lowering=False)

    xqt = nc.dram_tensor("xqt", [HID, SQC], fp32, kind="ExternalInput")
    xkvt = nc.dram_tensor("xkvt", [HID, S], fp32, kind="ExternalInput")
    maskf = nc.dram_tensor("maskf", [128, 16], fp32, kind="ExternalInput")
    wq = nc.dram_tensor("wq", [HID, HID], fp32, kind="ExternalInput")
    wk = nc.dram_tensor("wk", [HID, HID], fp32, kind="ExternalInput")
    wv = nc.dram_tensor("wv", [HID, HID], fp32, kind="ExternalInput")
    wo = nc.dram_tensor("wo", [HID, HID], fp32, kind="ExternalInput")
    y = nc.dram_tensor("y", [SQC, HID], fp32, kind="ExternalOutput")
    vspill = nc.dram_tensor("vspill", [NPAIR * S, 128], fp32, kind="Internal")

    # hand-drawn SBUF arena (fp32 word offsets per partition):
    #   KT  [0..16384)      K^T, pair-major: KT[p, t*2048+sk]
    #   QT  [16384..24576)  Q^T: QT[p, m*1024+sq]
    #   YPN [24576..32768)  normalized attn out^T: YPN[p, t*1024+sq]
    #   VP  [32768..40960)  2 bufs x 16 sk-tiles x [Va(64)|m(64)|Vb(64)|m(64)]
    arena = nc.alloc_sbuf_tensor("arena", [128, 40960], fp32)
    base = nc.lookup_mloc(arena).addr

    def at(name, words, off_words):
        return nc.alloc_sbuf_tensor_at(
            name, [128, words], fp32r, offset=base + off_words * 4
        )

    KT = at("KT", 16384, 0)
    QT = at("QT", 8192, 16384)
    YPN = at("YPN", 8192, 24576)
    VP = at("VP", 8192, 32768)
    # overlays
    WKs = at("WKs", 8192, 16384)     # QT region (phase A input)
    WVs = at("WVs", 8192, 24576)     # YPN region (phase A input)
    XKV0 = at("XKV0", 4096, 32768)   # VP region (phase A input)
    XKV1 = at("XKV1", 4096, 36864)
    XQs = at("XQs", 8192, 24576)     # YPN region (phase B input)
    WQs = at("WQs", 8192, 32768)     # VP region (phase B input)
    WOs = at("WOs", 8192, 0)         # KT region (phase D input)

    with tile.TileContext(nc) as tc, ExitStack() as ctx:
        misc = ctx.enter_context(tc.tile_pool(name="misc", bufs=1))
        pt_pool = ctx.enter_context(tc.tile_pool(name="ptp", bufs=3))
        vb_pool = ctx.enter_context(tc.tile_pool(name="vbp", bufs=3))
        rc_pool = ctx.enter_context(tc.tile_pool(name="rcp", bufs=2))
        ps_e = ctx.enter_context(tc.tile_pool(name="pse", bufs=3, space="PSUM"))
        ps_y = ctx.enter_context(tc.tile_pool(name="psy", bufs=2, space="PSUM"))

        maskt = misc.tile([128, 16], fp32)
        nc.sync.dma_start(maskt[:], maskf[:])

        # ---- Phase A: K^T and V (spilled to DRAM) ----
        for c in range(8):
            nc.sync.dma_start(WKs[:, c * 1024:(c + 1) * 1024],
                              wk[c * 128:(c + 1) * 128, :].bitcast(fp32r))
            nc.sync.dma_start(WVs[:, c * 1024:(c + 1) * 1024],
                              wv[c * 128:(c + 1) * 128, :].bitcast(fp32r))

        xkvbufs = [XKV0, XKV1]
        for q in range(4):
            buf = xkvbufs[q % 2]
            for c in range(8):
                nc.sync.dma_start(
                    buf[:, c * 512:(c + 1) * 512],
                    xkvt[c * 128:(c + 1) * 128, q * 512:(q + 1) * 512].bitcast(fp32r))
            # K^T -> KT
            for m in range(8):
                pk = ps_y.tile([128, 512], fp32, name="psyt")
                for c in range(8):
                    nc.tensor.matmul(
                        pk[:],
                        WKs[:, c * 1024 + m * 128: c * 1024 + (m + 1) * 128],
                        buf[:, c * 512:(c + 1) * 512],
                        start=(c == 0), stop=(c == 7))
                nc.vector.tensor_copy(
                    KT[:, m * 2048 + q * 512: m * 2048 + (q + 1) * 512],
                    pk[:].bitcast(fp32r))
            # V (masked) -> vspill
            for sl in range(4):
                sidx = q * 4 + sl
                for nv in range(2):
                    pv = ps_y.tile([128, 512], fp32, name="psyt")
                    for c in range(8):
                        nc.tensor.matmul(
                            pv[:],
                            buf[:, c * 512 + sl * 128: c * 512 + (sl + 1) * 128],
                            WVs[:, c * 1024 + nv * 512: c * 1024 + nv * 512 + 512],
                            start=(c == 0), stop=(c == 7))
                    vb = vb_pool.tile([128, 512], fp32r)
                    nc.vector.tensor_scalar_mul(
                        vb[:], pv[:].bitcast(fp32r), maskt[:, sidx:sidx + 1])
                    for pl in range(4):
                        t = nv * 4 + pl
                        nc.sync.dma_start(
                            vspill[t * 2048 + sidx * 128: t * 2048 + (sidx + 1) * 128, :]
                            .bitcast(fp32r),
                            vb[:, pl * 128:(pl + 1) * 128])

        # ---- Phase B: Q^T -> QT ----
        for c in range(8):
            nc.sync.dma_start(XQs[:, c * 1024:(c + 1) * 1024],
                              xqt[c * 128:(c + 1) * 128, :].bitcast(fp32r))
            nc.sync.dma_start(WQs[:, c * 1024:(c + 1) * 1024],
                              wq[c * 128:(c + 1) * 128, :].bitcast(fp32r))
        for m in range(8):
            for nq in range(2):
                pq = ps_y.tile([128, 512], fp32, name="psyt")
                for c in range(8):
                    nc.tensor.matmul(
                        pq[:],
                        WQs[:, c * 1024 + m * 128: c * 1024 + (m + 1) * 128],
                        XQs[:, c * 1024 + nq * 512: c * 1024 + nq * 512 + 512],
                        start=(c == 0), stop=(c == 7))
                nc.vector.tensor_copy(
                    QT[:, m * 1024 + nq * 512: m * 1024 + nq * 512 + 512],
                    pq[:].bitcast(fp32r))

        # ---- init VP mask columns (constant across pairs) ----
        ones = misc.tile([128, 64], fp32)
        nc.vector.memset(ones[:], 1.0)
        for bi in range(2):
            for s in range(16):
                for hh in range(2):
                    o = bi * 4096 + s * 256 + hh * 128 + 64
                    nc.vector.tensor_scalar_mul(
                        VP[:, o:o + 64], ones[:].bitcast(fp32r), maskt[:, s:s + 1])

        # ---- Phase C: attention per head pair ----
        with nc.allow_low_precision(reason="fp32r is full-width fp32"):
            for t in range(8):
                bi = t % 2
                vb_off = bi * 4096
                for s in range(16):
                    row = t * 2048 + s * 128
                    nc.sync.dma_start(
                        VP[:, vb_off + s * 256: vb_off + s * 256 + 64],
                        vspill[row:row + 128, 0:64].bitcast(fp32r))
                    nc.sync.dma_start(
                        VP[:, vb_off + s * 256 + 128: vb_off + s * 256 + 192],
                        vspill[row:row + 128, 64:128].bitcast(fp32r))
                for n in range(2):
                    for hh in range(2):
                        py = ps_y.tile([128, 512], fp32, name="psyt")
                        for sp in range(8):
                            pe = ps_e.tile([128, 1024], fp32)
                            for half in range(2):
                                s = sp * 2 + half
                                nc.tensor.matmul(
                                    pe[:, half * 512:(half + 1) * 512],
                                    KT[hh * 64:(hh + 1) * 64,
                                       t * 2048 + s * 128: t * 2048 + (s + 1) * 128],
                                    QT[hh * 64:(hh + 1) * 64,
                                       t * 1024 + n * 512: t * 1024 + n * 512 + 512],
                                    start=True, stop=True,
                                    tile_position=(hh * 64, 0))
                            pt = pt_pool.tile([128, 1024], fp32r)
                            nc.scalar.activation(pt[:], pe[:], Exp, scale=0.125)
                            for half in range(2):
                                s = sp * 2 + half
                                nc.tensor.matmul(
                                    py[:],
                                    VP[:, vb_off + s * 256 + hh * 128:
                                       vb_off + s * 256 + hh * 128 + 128],
                                    pt[:, half * 512:(half + 1) * 512],
                                    start=(sp == 0 and half == 0),
                                    stop=(sp == 7 and half == 1))
                        rc = rc_pool.tile([64, 512], fp32r)
                        nc.vector.reciprocal(rc[:], py[64:128, :].bitcast(fp32r))
                        nc.vector.tensor_mul(
                            YPN[hh * 64:(hh + 1) * 64,
                                t * 1024 + n * 512: t * 1024 + n * 512 + 512],
                            py[0:64, :].bitcast(fp32r), rc[:])

        # ---- Phase D: Y = YPN^T @ W_O ----
        for c in range(8):
            nc.sync.dma_start(WOs[:, c * 1024:(c + 1) * 1024],
                              wo[c * 128:(c + 1) * 128, :].bitcast(fp32r))
        for m in range(8):
            for no in range(2):
                pd = ps_y.tile([128, 512], fp32, name="psyt")
                for tt in range(8):
                    nc.tensor.matmul(
                        pd[:],
                        YPN[:, tt * 1024 + m * 128: tt * 1024 + (m + 1) * 128],
                        WOs[:, tt * 1024 + no * 512: tt * 1024 + no * 512 + 512],
                        start=(tt == 0), stop=(tt == 7))
                ob = vb_pool.tile([128, 512], fp32)
                nc.vector.tensor_copy(ob[:], pd[:])
                nc.sync.dma_start(
                    y[m * 128:(m + 1) * 128, no * 512: no * 512 + 512], ob[:])

    nc.finalize()
    return nc


def _get_program():
    global _PROG
    if _PROG is None:
        _PROG = _build_program()
    return _PROG


def _make_in_maps(inputs):
    X_Q = np.ascontiguousarray(np.asarray(inputs["X_Q"], dtype=np.float32))
    X_KV = np.ascontiguousarray(np.asarray(inputs["X_KV"], dtype=np.float32))
    mask = np.asarray(inputs["key_padding_mask"])
    W_Q = np.ascontiguousarray(np.asarray(inputs["W_Q"], dtype=np.float32))
    W_K = np.ascontiguousarray(np.asarray(inputs["W_K"], dtype=np.float32))
    W_V = np.ascontiguousarray(np.asarray(inputs["W_V"], dtype=np.float32))
    W_O = np.ascontiguousarray(np.asarray(inputs["W_O"], dtype=np.float32))
    in_maps = []
    for core in range(8):
        b, half = core // 2, core % 2
        xqt = np.ascontiguousarray(X_Q[b, half * SQC:(half + 1) * SQC, :].T)
        xkvt = np.ascontiguousarray(X_KV[b].T)
        keep = (~mask[b].astype(bool)).astype(np.float32)
        maskf = np.ascontiguousarray(keep.reshape(16, 128).T)
        in_maps.append({
            "xqt": xqt, "xkvt": xkvt, "maskf": maskf,
            "wq": W_Q, "wk": W_K, "wv": W_V, "wo": W_O,
        })
    return in_maps


def kernel(**inputs):
    nc = _get_program()
    in_maps = _make_in_maps(inputs)
    res = run_bass_kernel_spmd(nc, in_maps, core_ids=list(range(8)))
    out = np.empty((B, S, HID), dtype=np.float32)
    for core in range(8):
        b, half = core // 2, core % 2
        out[b, half * SQC:(half + 1) * SQC, :] = res.results[core]["y"]
    return out
